# revision 14
# baseline (speedup 1.0000x reference)
"""Trainium2 Bass kernel for nn_ClauseInferModule (gnn_message_passing).

out[c, b, g] = sum_s prod_l x[b, I[c, g, s, l]],  B=16 G=16384 C=8 S=8 L=4.

Sharding: clause-per-core (C == n_cores == 8). Per core the 524288 random
lookups run on the SWDGE dma_gather path instead of GPSIMD ap_gather:

  - the valuation table sits in SBUF as xs[p, r, b] = x[b, 128r+p] (1MB);
    one descriptor fetches the 64B 16-batch vector of one atom directly into
    the consumer layout [128 positions, W cols, 16 b] -- no extraction pass,
  - 64 dma_gather calls of 8192 indices each (one per (s, g-half, l); 8192*4B
    of Q7 index scratch stays under the 64KB SCRATCH_BUF_SIZE). Descriptor
    GENERATION on each queue's Q7 core pair is the bottleneck (~8.3 ns/desc
    measured); 4 queues keep all 8 Q7 cores generating concurrently, ~1.6x
    the all-core ap_gather rate, with the SDMA engines well under capacity.
    single_packet=False: concatenating 512 descriptors into one SDMA packet
    exceeds the packet ceiling and wedges the device,
  - index tiles rotate through 8 slots; each [128, 2048] group tile holds
    the four l-calls' wrapped index lists (replicated across the 8 Q7 core
    groups, as the dma_gather contract requires),
  - VectorE multiplies the four l-streams and accumulates over s,
  - one DMA per g-half writes the [128, 64, 16] accumulator to HBM; the host
    undoes the (p, h, w, b) -> (b, g) layout.

dma_gather's bass wrapper insists on elem_size % 256B == 0 (a transpose-mode
restriction); the non-transpose ucode only needs the row *stride* 256B-aligned,
so the instruction is emitted directly with elem_size=16 f32 (64B payload).
"""
import os
import sys
import numpy as np

sys.path.insert(0, "/opt/trn_rl_repo")

import concourse.bacc as bacc
import concourse.bass as bass
from concourse import mybir
from concourse.bass_utils import run_bass_kernel_spmd
from concourse.library_config import mlp

B, G = 16, 16384
C, S, L = 8, 8, 4
W = 64                # columns per gather call
NIDX = 128 * W        # 8192 indices per dma_gather call (one (s, half, l))
H = 2                 # g-halves per s
NGROUP = S * H        # 16 (s, half) groups
NCALL = NGROUP * L    # 64 calls per core
NQ = 4                # SWDGE queues (ucode max): queue l's Q7 core pair
                      # generates descriptors concurrently with the others
NSUB = 1              # dma_gather subcalls per (group, l)
KSP = False           # single_packet concat trips the SDMA packet ceiling
RSETS = 2             # rotating gather-buffer sets (s parity)
IDX_COLS = NIDX // 16      # 512 wrapped idx columns per call
GRP_COLS = L * IDX_COLS    # 2048 columns per (s, half) group tile
IDX_SLOTS = 8              # rotating group-tile slots

_compiled = None
last_exec_time_ns = None


def _emit_dma_gather(gp, out_ap, in_ap, idxs_ap, num_idxs, elem_size,
                     elem_step, queue_num, src_sbuf=False):
    """mybir.InstDMAGatherAnt emit, mirroring BassGpSimd.dma_gather but
    without the transpose-only elem_size%256B restriction and allowing an
    SBUF source in non-transpose mode (the ucode's gen_descs handles both;
    only the bass wrapper and the interp restrict them)."""
    assert in_ap.ap[-1][1] == out_ap.ap[-1][1] == elem_size
    assert out_ap.ap[0][1] * out_ap.ap[1][1] == num_idxs
    assert num_idxs * 4 + 1024 < (1 << 16) - 64  # Q7 scratch buffer limit
    if src_sbuf:
        stride_bytes_256 = 0
        sbuf_kw = dict(sbuf_tokens_per_rank=128,
                       sbuf_free_dim_per_rank=elem_size * 4,
                       sbuf_free_dim_pad_per_rank=0, sbuf_byte_offset=0)
        _in_ap = [gp.lower_ap(in_ap)]
    else:
        stride_bytes = elem_step * mybir.dt.size(in_ap.dtype)
        assert stride_bytes % 256 == 0 and stride_bytes // 256 < 256
        assert in_ap.ap[0][0] == elem_step
        stride_bytes_256 = stride_bytes // 256
        sbuf_kw = {}
        _in_ap = gp.lower_ap_dma(in_ap, for_custom_bir_dma=True)
    inst = gp.add_instruction(
        mybir.InstDMAGatherAnt(
            name=gp.bass.get_next_instruction_name(),
            ins=[
                *_in_ap,
                gp.lower_ap(idxs_ap),
                gp.lower_val_access(gp.to_reg(num_idxs)),
            ],
            outs=[gp.lower_ap(out_ap)],
            transpose=False,
            num_idxs=num_idxs,
            elem_size=elem_size,
            stride_bytes_256=stride_bytes_256,
            gen_mode=0,
            single_packet=KSP,
            queue_num=queue_num,
            **sbuf_kw,
        )
    )
    return inst


def _build(src_sbuf: bool = False):
    nc = bacc.Bacc("TRN2", target_bir_lowering=False, debug=False,
                   num_swdge_queues=NQ, dynamic_dma_scratch_size=32768)
    if src_sbuf:
        xt_d = nc.dram_tensor("xt", [128, G // 128, 16], mybir.dt.float32,
                              kind="ExternalInput")
    else:
        xt_d = nc.dram_tensor("xt", [G, 64], mybir.dt.float32,
                              kind="ExternalInput")
    idx_d = nc.dram_tensor("idx", [128, NGROUP * GRP_COLS], mybir.dt.int16,
                           kind="ExternalInput")
    out_d = nc.dram_tensor("out", [128, H, W, 16], mybir.dt.float32,
                           kind="ExternalOutput")

    from contextlib import ExitStack
    with ExitStack() as ctx:
        block = ctx.enter_context(nc.Block())
        bufs = [[[ctx.enter_context(
                     nc.sbuf_tensor(f"buf_{r}_{h}_{l}", [128, W, 16],
                                    mybir.dt.float32))
                  for l in range(L)] for h in range(H)] for r in range(RSETS)]
        idxt = [ctx.enter_context(
                    nc.sbuf_tensor(f"idx_{j}", [128, GRP_COLS],
                                   mybir.dt.int16))
                for j in range(IDX_SLOTS)]
        t1 = ctx.enter_context(
            nc.sbuf_tensor("t1", [128, W, 16], mybir.dt.float32))
        t2 = ctx.enter_context(
            nc.sbuf_tensor("t2", [128, W, 16], mybir.dt.float32))
        t3 = ctx.enter_context(
            nc.sbuf_tensor("t3", [128, W, 16], mybir.dt.float32))
        acc = [ctx.enter_context(
                   nc.sbuf_tensor(f"acc{h}", [128, W, 16], mybir.dt.float32))
               for h in range(H)]
        xs = (ctx.enter_context(
                  nc.sbuf_tensor("xs", [128, G // 128, 16], mybir.dt.float32))
              if src_sbuf else None)
        # One outstanding DMA per semaphore => cumulative waits are exact.
        idx_sem = [ctx.enter_context(nc.semaphore(f"idx_sem{j}"))
                   for j in range(IDX_SLOTS)]
        gat_sem = [[[ctx.enter_context(nc.semaphore(f"gat{q}_{r}_{h}"))
                     for h in range(H)] for r in range(RSETS)]
                   for q in range(L)]
        dve_sem = ctx.enter_context(nc.semaphore("dve_sem"))
        vchain = ctx.enter_context(nc.semaphore("vchain"))
        out_sem = ctx.enter_context(nc.semaphore("out_sem"))
        xs_sem = ctx.enter_context(nc.semaphore("xs_sem"))

        @block.sync
        def _(sync):
            if src_sbuf:
                sync.dma_start(xs[:, :, :], xt_d[:, :, :]).then_inc(xs_sem, 16)
            for g in range(NGROUP):
                if g >= IDX_SLOTS:
                    # slot free once all 4 gathers of group g-IDX_SLOTS ran
                    gp_, hp = divmod(g - IDX_SLOTS, H)
                    for q in range(L):
                        sync.wait_ge(gat_sem[q][gp_ % RSETS][hp],
                                     16 * NSUB * (gp_ // RSETS + 1))
                sync.dma_start(
                    idxt[g % IDX_SLOTS][:, :],
                    idx_d[:, g * GRP_COLS:(g + 1) * GRP_COLS],
                ).then_inc(idx_sem[g % IDX_SLOTS], 16)
            sync.wait_ge(dve_sem, NGROUP)
            for h in range(H):
                sync.dma_start(out_d[:, h, :, :], acc[h][:, :, :]) \
                    .then_inc(out_sem, 16)
            sync.wait_ge(out_sem, 16 * H)

        @block.gpsimd
        def _(gp):
            gp.load_library(mlp)
            if src_sbuf:
                gp.wait_ge(xs_sem, 16)
            for k in range(NCALL):
                g, l = divmod(k, L)
                s, h = divmod(g, H)
                r = s % RSETS
                if l == 0:
                    gp.wait_ge(idx_sem[g % IDX_SLOTS],
                               16 * (g // IDX_SLOTS + 1))
                    if s >= RSETS:
                        # buffer set free once DVE consumed group (s-RSETS, h)
                        gp.wait_ge(dve_sem, (s - RSETS) * H + h + 1)
                ni = NIDX // NSUB
                nc_cols = IDX_COLS // NSUB
                nw = W // NSUB
                for u in range(NSUB):
                    _emit_dma_gather(
                        gp,
                        out_ap=bufs[r][h][l][:, u * nw:(u + 1) * nw, :],
                        in_ap=xs[:, :, :] if src_sbuf else xt_d[:, 0:16],
                        idxs_ap=idxt[g % IDX_SLOTS][:,
                            l * IDX_COLS + u * nc_cols:
                            l * IDX_COLS + (u + 1) * nc_cols],
                        num_idxs=ni,
                        elem_size=16,
                        elem_step=64,
                        queue_num=l % NQ,
                        src_sbuf=src_sbuf,
                    ).then_inc(gat_sem[l][r][h], 16)

        @block.vector
        def _(vec):
            # DVE executes in order, but raw-block mode has no implicit
            # dependency tracking: serialize the stream through vchain /
            # dve_sem (one sem update per instruction).
            nv, nd = 0, 0

            def op(final, f, *args):
                nonlocal nv, nd
                if nv:
                    vec.wait_ge(vchain, nv)
                if nd:
                    vec.wait_ge(dve_sem, nd)
                inst = f(*args)
                if final:
                    inst.then_inc(dve_sem, 1)
                    nd += 1
                else:
                    inst.then_inc(vchain, 1)
                    nv += 1
                return inst

            for g in range(NGROUP):
                s, h = divmod(g, H)
                r = s % RSETS
                for q in range(L):
                    vec.wait_ge(gat_sem[q][r][h],
                                16 * NSUB * (s // RSETS + 1))
                v = bufs[r][h]
                a = acc[h]
                op(0, vec.tensor_mul, t1[:, :, :], v[0][:, :, :], v[1][:, :, :])
                op(0, vec.tensor_mul, t2[:, :, :], v[2][:, :, :], v[3][:, :, :])
                if s == 0:
                    op(1, vec.tensor_mul, a[:, :, :], t1[:, :, :], t2[:, :, :])
                else:
                    op(0, vec.tensor_mul, t3[:, :, :], t1[:, :, :], t2[:, :, :])
                    op(1, vec.tensor_add, a[:, :, :], a[:, :, :], t3[:, :, :])

    nc.compile()
    return nc


def _prep_inputs(x: np.ndarray, I: np.ndarray, src_sbuf: bool = False):
    """Host-side layout transforms: padded transposed table + wrapped int16
    index streams. Group tile (s, h) packs the four l-calls' index lists in
    32-partition bands (each band: wrapped-in-16 layout, duplicated for the
    queue's TX and RX Q7 cores)."""
    if src_sbuf:
        # xs[p, r, b] = x[b, 128*r + p]
        xt = np.ascontiguousarray(
            np.transpose(x.reshape(B, G // 128, 128), (2, 1, 0)))
    else:
        xt = np.zeros((G, 64), np.float32)
        xt[:, 0:16] = x.T
    feeds = []
    for c in range(C):
        arr = np.empty((128, NGROUP * GRP_COLS), np.int16)
        for g in range(NGROUP):
            s, h = divmod(g, H)
            for l in range(L):
                v = I[c, h * NIDX:(h + 1) * NIDX, s, l].astype(np.int16)
                w = v.reshape(IDX_COLS, 16).T          # wrapped [16, 512]
                arr[:, g * GRP_COLS + l * IDX_COLS:
                    g * GRP_COLS + (l + 1) * IDX_COLS] = np.tile(w, (8, 1))
        feeds.append(arr)
    return xt, feeds


SRC_SBUF = True       # SBUF-resident table measured ~2% faster than
                      # HBM-source (1066us vs 1088us per-core exec)


def kernel(x: np.ndarray, I: np.ndarray) -> np.ndarray:
    global _compiled, last_exec_time_ns
    if _compiled is None:
        _compiled = _build(src_sbuf=SRC_SBUF)
    nc = _compiled

    x = np.ascontiguousarray(np.asarray(x), dtype=np.float32)
    xt, idx_feeds = _prep_inputs(x, np.asarray(I), src_sbuf=SRC_SBUF)

    in_maps = [{"xt": xt, "idx": idx_feeds[c]} for c in range(C)]
    kwargs = {}
    if os.environ.get("KERNEL_TRACE") == "1":
        kwargs = {"trace": True, "trace_cores": list(range(C))}
    res = run_bass_kernel_spmd(nc, in_maps, core_ids=list(range(C)), **kwargs)
    last_exec_time_ns = res.exec_time_ns
    # res[c]["out"][p, h, w, b] = out[c, b, h*8192 + w*128 + p]
    out = np.empty((C, B, G), np.float32)
    for c in range(C):
        r = np.asarray(res.results[c]["out"]).reshape(128, H, W, 16)
        out[c] = r.transpose(3, 1, 2, 0).reshape(B, G)
    return np.ascontiguousarray(out)


if __name__ == "__main__":
    rng = np.random.default_rng(0)
    x = rng.random((B, G), dtype=np.float32)
    I = rng.integers(0, G, size=(C, G, S, L)).astype(np.int64)
    out = kernel(x=x, I=I)
    gathered = x[:, I]
    expect = np.moveaxis(np.sum(np.prod(gathered, axis=-1), axis=-1), 0, 1)
    err = np.abs(out - expect).max() / np.abs(expect).max()
    print("max rel err:", err)
